# revision 27
# baseline (speedup 1.0000x reference)
"""Trainium2 Bass kernel for nn_DeepQNetIVCML (GNN message passing).

Reference computation per (b, a) pair:
  multi-hop coverage over a sparse binary adjacency (3 steps), weighted
  feature aggregation, mask + mean-normalize, then a small shared MLP.

Sharding: 128 (b, a) pairs split across 8 cores (16 pairs each; every
core sees exactly one b). MLP weights are replicated.

Key kernel ideas:
  - Propagation runs in "path count" space: p_{t+1} = A^T p_t with no
    thresholding between steps (support(p_t) is exact under any
    non-negative rounding), so cover_t = min(prefix_sum, 1) and the
    per-node weight is a telescoped linear combination of covers.
  - Adjacency and seed vectors are binary -> exact in bf16. A-stationary
    matmuls keep the state in column layout (no transposes in the loop).
  - fea = F^T w streams fea_emb as the moving operand in bf16 hi+lo
    halves (F = Fhi + Flo, both bf16) for ~1e-5 relative accuracy. The
    per-node weights divided by ALPHA^4 are exact dyadic rationals
    (ALPHA = 0.8 -> {1.953125, 1.5625, 1.25, 1}), exact in bf16; ALPHA^4
    folds into the per-pair scalar.
  - mask/denominator/ALPHA^4 fold into one per-pair scalar applied via
    the activation scale at the relu.
"""

import os
import sys

for _p in ("/opt/trn_rl_repo", "/opt/pypackages"):
    if os.path.isdir(_p) and _p not in sys.path:
        sys.path.insert(0, _p)

import ml_dtypes
import numpy as np

import concourse.bacc as bacc
import concourse.mybir as mybir
from concourse import masks
from concourse.bass_utils import run_bass_kernel_spmd
from concourse.tile import TileContext

B, A, N, D, L = 4, 32, 512, 768, 128
ALPHA = 0.8
STEP_NUM = 3
NCORES = 8
P_PER = (B * A) // NCORES  # pairs per core
NCH = N // 128             # node chunks
DG = D // 128              # feature chunks

BF16 = mybir.dt.bfloat16
F8 = mybir.dt.float8e4
F32 = mybir.dt.float32
F32R = mybir.dt.float32r
BF16_NP = ml_dtypes.bfloat16
F8_NP = ml_dtypes.float8_e4m3

_PROG = None
LAST_RESULT = None


def _build():
    nc = bacc.Bacc("TRN2", target_bir_lowering=False, debug=False,
                   num_devices=NCORES)

    a_pre = nc.dram_tensor("a_pre", [128, P_PER * NCH * N], F8,
                           kind="ExternalInput")
    fhi_pre = nc.dram_tensor("fhi_pre", [128, P_PER * NCH * D], BF16,
                             kind="ExternalInput")
    flo_pre = nc.dram_tensor("flo_pre", [128, P_PER * NCH * D], BF16,
                             kind="ExternalInput")
    s0_pre = nc.dram_tensor("s0_pre", [128, P_PER * NCH], F8,
                            kind="ExternalInput")
    mask_pre = nc.dram_tensor("mask_pre", [1, P_PER], F32,
                              kind="ExternalInput")
    q_pre = nc.dram_tensor("q_pre", [L, D], F32, kind="ExternalInput")
    w1_pre = nc.dram_tensor("w1_pre", [128, DG * D], F32,
                            kind="ExternalInput")
    w2_pre = nc.dram_tensor("w2_pre", [128, 2 * DG * D], F32,
                            kind="ExternalInput")
    w3_pre = nc.dram_tensor("w3_pre", [128, DG], F32, kind="ExternalInput")
    b1_pre = nc.dram_tensor("b1_pre", [1, D], F32, kind="ExternalInput")
    b2_pre = nc.dram_tensor("b2_pre", [1, D], F32, kind="ExternalInput")
    b3_pre = nc.dram_tensor("b3_pre", [1, 1], F32, kind="ExternalInput")
    y_out = nc.dram_tensor("y", [P_PER, 1], F32, kind="ExternalOutput")
    warm_out = nc.dram_tensor("warm", [1, 1], F32, kind="ExternalOutput")

    mult = mybir.AluOpType.mult
    add = mybir.AluOpType.add
    relu = mybir.ActivationFunctionType.Relu

    # per-cover weights scaled by ALPHA^-4: exact dyadic rationals
    c_init = 1.0 / ALPHA**3 - 1.0 / ALPHA**2       # 0.390625
    coefs = [1.0 / ALPHA**2 - 1.0 / ALPHA,         # 0.3125
             1.0 / ALPHA - 1.0,                    # 0.25
             1.0]
    a4 = float(np.float32(ALPHA) ** 4)

    with TileContext(nc) as tc:
        with (
            tc.tile_pool(name="const", bufs=1) as cpool,
            tc.tile_pool(name="weights", bufs=1) as wpool,
            tc.tile_pool(name="abuf", bufs=6) as apool,
            tc.tile_pool(name="fbuf", bufs=6) as fpool,
            tc.tile_pool(name="small", bufs=4) as spool,
            tc.tile_pool(name="nfrow", bufs=2) as nfpool,
            tc.tile_pool(name="mlp", bufs=1) as mpool,
        ):
            ident = cpool.tile([128, 128], F32)
            masks.make_identity(nc, ident[:])
            ones16 = cpool.tile([1, P_PER], F32)
            nc.vector.memset(ones16[:], 1.0)
            onesL = cpool.tile([128, 1], F32)
            nc.vector.memset(onesL[:], 1.0 / L)
            ones128 = cpool.tile([128, 1], F32)
            nc.vector.memset(ones128[:], 1.0)

            s0_sb = cpool.tile([128, P_PER * NCH], F8)
            nc.sync.dma_start(s0_sb[:], s0_pre[:])
            mask_sb = cpool.tile([1, P_PER], F32)
            nc.sync.dma_start(mask_sb[:], mask_pre[:])

            def pair_dmas(p):
                A_sb = apool.tile([128, NCH * N], F8, tag="A")
                nc.sync.dma_start(A_sb[:],
                                  a_pre[:, p * NCH * N:(p + 1) * NCH * N])
                Fhi_sb = fpool.tile([128, NCH * D], BF16, tag="Fhi")
                nc.sync.dma_start(Fhi_sb[:],
                                  fhi_pre[:, p * NCH * D:(p + 1) * NCH * D])
                Flo_sb = fpool.tile([128, NCH * D], BF16, tag="Flo")
                nc.sync.dma_start(Flo_sb[:],
                                  flo_pre[:, p * NCH * D:(p + 1) * NCH * D])
                return A_sb, Fhi_sb, Flo_sb

            # issue ALL pair-data DMAs first: the SP HWDGE ring is FIFO, so
            # anything else placed between them delays the streaming phase.
            # MLP weights ride at the END of the ring — they're only needed
            # once every pair has been aggregated.
            staged = {p: pair_dmas(p) for p in range(P_PER)}

            # weights go on the ACT HWDGE ring: a separate FIFO that
            # round-robins with the SP ring at the SDMA engines, so they
            # arrive early without delaying the bulk pair stream
            q_sb = cpool.tile([L, D], F32)
            nc.scalar.dma_start(q_sb[:], q_pre[:])
            w1_sb = wpool.tile([128, DG * D], F32)
            nc.scalar.dma_start(w1_sb[:], w1_pre[:])
            w2_sb = wpool.tile([128, 2 * DG * D], F32)
            nc.scalar.dma_start(w2_sb[:, 0:DG * D], w2_pre[:, 0:DG * D])
            nc.scalar.dma_start(w2_sb[:, DG * D:], w2_pre[:, DG * D:])
            w3_sb = wpool.tile([128, DG], F32)
            nc.scalar.dma_start(w3_sb[:], w3_pre[:])
            b1_sb = cpool.tile([1, D], F32)
            nc.scalar.dma_start(b1_sb[:], b1_pre[:])
            b2_sb = cpool.tile([1, D], F32)
            nc.scalar.dma_start(b2_sb[:], b2_pre[:])
            b3_sb = cpool.tile([1, 1], F32)
            nc.scalar.dma_start(b3_sb[:], b3_pre[:])

            nf16 = mpool.tile([P_PER, D], F32)

            with (
                tc.tile_pool(name="ppps", bufs=2, space="PSUM") as pp_psum,
                tc.tile_pool(name="feaps", bufs=1, space="PSUM") as fea_psum,
                tc.tile_pool(name="denps", bufs=1, space="PSUM") as den_psum,
                tc.tile_pool(name="qps", bufs=1, space="PSUM") as qpsum,
            ):
                qb2 = mpool.tile([1, D], F32)

                def q_block():
                    # q-side of the MLP: placed mid-loop in PE program
                    # order so its weight-DMA waits never head-block the
                    # PE instruction FIFO
                    qT = mpool.tile([128, DG], F32)
                    qtp = qpsum.tile([128, 1], F32, tag="qt")
                    for g in range(DG):
                        nc.tensor.matmul(qtp[:],
                                         q_sb[:, g * 128:(g + 1) * 128],
                                         onesL[:], start=True, stop=True)
                        nc.scalar.copy(qT[:, g:g + 1], qtp[:])
                    qwp = qpsum.tile([1, D], F32, tag="qw")
                    for lo, hi in ((0, 512), (512, D)):
                        for g in range(DG):
                            nc.tensor.matmul(
                                qwp[:, lo:hi],
                                qT[:, g:g + 1],
                                w2_sb[:, (DG + g) * D + lo:(DG + g) * D + hi],
                                start=(g == 0), stop=(g == DG - 1))
                    nc.vector.tensor_add(qb2[:], qwp[:], b2_sb[:])

                # two pairs interleaved: pair a's matmuls fill the PE
                # bubbles left by pair b's DVE dependency chain
                for pp in range(0, P_PER, 2):
                    duo = (pp, pp + 1)
                    st = {}
                    for p in duo:
                        A_sb, Fhi_sb, Flo_sb = staged.pop(p)
                        s0c = s0_sb[:, p * NCH:(p + 1) * NCH]
                        pcur = spool.tile([128, NCH], F8, tag="pcur")
                        nc.vector.tensor_copy(pcur[:], s0c)
                        pref = spool.tile([128, NCH], F32, tag="pref")
                        nc.vector.tensor_copy(pref[:], s0c)
                        wcol = spool.tile([128, NCH], F32, tag="wcol")
                        nc.vector.tensor_scalar_mul(wcol[:], pref[:], c_init)
                        ct = spool.tile([128, NCH], F32, tag="ct")
                        st[p] = dict(A=A_sb, Fhi=Fhi_sb, Flo=Flo_sb,
                                     pcur=pcur, pref=pref, wcol=wcol, ct=ct)

                    for t in range(STEP_NUM):
                        for p in duo:
                            s = st[p]
                            ps = pp_psum.tile([128, NCH], F32, tag="pp")
                            s["ps"] = ps
                            for oc in range(NCH):
                                base = oc * 128
                                for ic in range(NCH):
                                    nc.tensor.matmul(
                                        ps[:, oc:oc + 1],
                                        s["A"][:, ic * N + base:
                                               ic * N + base + 128],
                                        s["pcur"][:, ic:ic + 1],
                                        start=(ic == 0),
                                        stop=(ic == NCH - 1),
                                    )
                        for p in duo:
                            s = st[p]
                            ps = s["ps"]
                            # clamp to {0,1} so the fp8 cast is exact (e4m3
                            # overflows to inf above 240; counts can exceed)
                            pnext = spool.tile([128, NCH], F8, tag="pcur")
                            nc.vector.tensor_scalar_min(pnext[:], ps[:], 1.0)
                            nc.vector.tensor_add(s["pref"][:], s["pref"][:],
                                                 ps[:])
                            nc.vector.tensor_scalar_min(s["ct"][:],
                                                        s["pref"][:], 1.0)
                            nc.vector.scalar_tensor_tensor(
                                s["wcol"][:], s["ct"][:], coefs[t],
                                s["wcol"][:], op0=mult, op1=add)
                            s["pcur"] = pnext

                    for p in duo:
                        s = st[p]
                        dps = den_psum.tile([1, NCH], F32, tag="den")
                        nc.tensor.matmul(dps[:], ones128[:], s["ct"][:],
                                         start=True, stop=True)
                        den = spool.tile([1, 1], F32, tag="dens")
                        nc.vector.tensor_reduce(den[:], dps[:],
                                                axis=mybir.AxisListType.X,
                                                op=add)
                        # coverage count is an integer >= 1 unless the seed
                        # set is empty (w == 0 there, so any scale gives 0)
                        nc.vector.tensor_scalar_max(den[:], den[:], 0.5)
                        rec = spool.tile([1, 1], F32, tag="rec")
                        nc.vector.reciprocal(rec[:], den[:])
                        inv = spool.tile([1, 1], F32, tag="inv")
                        # fold mask and the ALPHA^4 rescale into one scalar
                        nc.vector.scalar_tensor_tensor(
                            inv[:], rec[:], a4, mask_sb[:, p:p + 1],
                            op0=mult, op1=mult)
                        s["inv"] = inv
                        ubf = spool.tile([128, NCH], BF16, tag="ubf")
                        nc.vector.tensor_copy(ubf[:], s["wcol"][:])
                        s["ubf"] = ubf

                    for p in duo:
                        s = st[p]
                        fps = fea_psum.tile([1, D], F32, tag="fea")
                        for lo, hi in ((0, 512), (512, D)):
                            for ci, half in enumerate(
                                    [(c, h) for c in range(NCH)
                                     for h in (s["Fhi"], s["Flo"])]):
                                c, hsb = half
                                nc.tensor.matmul(
                                    fps[:, lo:hi],
                                    s["ubf"][:, c:c + 1],
                                    hsb[:, c * D + lo:c * D + hi],
                                    start=(ci == 0),
                                    stop=(ci == 2 * NCH - 1),
                                )
                        nfr = nfpool.tile([1, D], F32, tag="nfr")
                        nc.scalar.activation(nfr[:], fps[:], relu,
                                             scale=s["inv"][:])
                        # ACT ring: keeps the SP ring free for bulk streaming
                        nc.scalar.dma_start(nf16[p:p + 1, :], nfr[:])

                    if pp == 6:
                        q_block()

                # keep the PE array's HAM clock warm across the wait for
                # the last nf row, so the fp32 MLP streams run at 2.4GHz
                last_A = st[P_PER - 1]["A"]
                wps = fea_psum.tile([1, D], F32, tag="fea")
                for _ in range(12):
                    nc.tensor.matmul(wps[:, 0:512], s0_sb[:, 0:1],
                                     last_A[:, 0:512], start=True, stop=True)
                warm_sb = spool.tile([1, 1], F32, tag="dens")
                nc.vector.tensor_copy(warm_sb[:], wps[0:1, 0:1])
                nc.scalar.dma_start(warm_out[:], warm_sb[:])

            with (
                tc.tile_pool(name="mlpps", bufs=1, space="PSUM") as mps,
                tc.tile_pool(name="trps", bufs=2, space="PSUM") as tps,
            ):
                def transp(src_sb, dst_sb):
                    for g in range(DG):
                        tp = tps.tile([128, P_PER], F32, tag="tr")
                        nc.tensor.matmul(tp[:], src_sb[:, g * 128:(g + 1) * 128],
                                         ident[0:P_PER, 0:P_PER],
                                         is_transpose=True)
                        nc.scalar.copy(dst_sb[:, g * P_PER:(g + 1) * P_PER],
                                       tp[:])

                nfT = mpool.tile([128, DG * P_PER], F32)
                transp(nf16, nfT)

                h1 = mpool.tile([P_PER, D], F32)
                hp = mps.tile([P_PER, D], F32, tag="h")
                for lo, hi in ((0, 512), (512, D)):
                    for g in range(DG):
                        nc.tensor.matmul(
                            hp[:, lo:hi],
                            nfT[:, g * P_PER:(g + 1) * P_PER],
                            w1_sb[:, g * D + lo:g * D + hi],
                            start=(g == 0), stop=False)
                    nc.tensor.matmul(hp[:, lo:hi], ones16[:],
                                     b1_sb[:, lo:hi],
                                     start=False, stop=True)
                nc.scalar.activation(h1[:], hp[:], relu)

                h1T = mpool.tile([128, DG * P_PER], F32)
                transp(h1, h1T)

                h2 = mpool.tile([P_PER, D], F32)
                hp2 = mps.tile([P_PER, D], F32, tag="h")
                for lo, hi in ((0, 512), (512, D)):
                    for g in range(DG):
                        nc.tensor.matmul(
                            hp2[:, lo:hi],
                            h1T[:, g * P_PER:(g + 1) * P_PER],
                            w2_sb[:, g * D + lo:g * D + hi],
                            start=(g == 0), stop=False)
                    nc.tensor.matmul(hp2[:, lo:hi], ones16[:],
                                     qb2[:, lo:hi],
                                     start=False, stop=True)
                nc.scalar.activation(h2[:], hp2[:], relu)

                h2T = mpool.tile([128, DG * P_PER], F32)
                transp(h2, h2T)

                yp = tps.tile([128, P_PER], F32, tag="tr")
                for g in range(DG):
                    nc.tensor.matmul(yp[0:P_PER, 0:1],
                                     h2T[:, g * P_PER:(g + 1) * P_PER],
                                     w3_sb[:, g:g + 1],
                                     start=(g == 0), stop=False)
                nc.tensor.matmul(yp[0:P_PER, 0:1], ones16[:], b3_sb[:],
                                 start=False, stop=True)
                ysb = mpool.tile([P_PER, 1], F32)
                nc.vector.tensor_copy(ysb[:], yp[0:P_PER, 0:1])
                nc.scalar.dma_start(y_out[:], ysb[:])

    nc.compile()
    return nc


def get_program():
    global _PROG
    if _PROG is None:
        _PROG = _build()
    return _PROG


def _prep_core(core, query_fea, a_nei, vec_nei, fea_emb, nei_mask,
               W1, b1, W2, b2, W3, b3):
    b = (core * P_PER) // A
    a0 = (core * P_PER) % A
    a_loc = a_nei[b, a0:a0 + P_PER]
    f_loc = fea_emb[b, a0:a0 + P_PER]
    s_loc = vec_nei[b, a0:a0 + P_PER]
    f_chunked = np.ascontiguousarray(
        f_loc.reshape(P_PER, NCH, 128, D).transpose(2, 0, 1, 3)
        .reshape(128, P_PER * NCH * D)).astype(np.float32)
    f_hi = f_chunked.astype(BF16_NP)
    f_lo = (f_chunked - f_hi.astype(np.float32)).astype(BF16_NP)
    return {
        "a_pre": np.ascontiguousarray(
            a_loc.reshape(P_PER, NCH, 128, N).transpose(2, 0, 1, 3)
            .reshape(128, P_PER * NCH * N)).astype(F8_NP),
        "fhi_pre": f_hi,
        "flo_pre": f_lo,
        "s0_pre": np.ascontiguousarray(
            s_loc.reshape(P_PER, NCH, 128).transpose(2, 0, 1)
            .reshape(128, P_PER * NCH)).astype(F8_NP),
        "mask_pre": nei_mask[b, a0:a0 + P_PER, 0].reshape(1, P_PER)
        .astype(np.float32),
        "q_pre": query_fea[b].astype(np.float32),
        "w1_pre": np.ascontiguousarray(
            W1.reshape(DG, 128, D).transpose(1, 0, 2).reshape(128, DG * D))
        .astype(np.float32),
        "w2_pre": np.ascontiguousarray(
            W2.reshape(2 * DG, 128, D).transpose(1, 0, 2)
            .reshape(128, 2 * DG * D)).astype(np.float32),
        "w3_pre": np.ascontiguousarray(
            W3[:, 0].reshape(DG, 128).transpose(1, 0)).astype(np.float32),
        "b1_pre": b1.reshape(1, D).astype(np.float32),
        "b2_pre": b2.reshape(1, D).astype(np.float32),
        "b3_pre": b3.reshape(1, 1).astype(np.float32),
    }


_EXEC = None


def _make_exec():
    """Replicates bass2jax.run_bass_via_pjrt's multi-core path, but caches
    the jitted executable so repeated calls (and timing loops) skip
    recompilation."""
    global _EXEC
    if _EXEC is not None:
        return _EXEC
    import jax
    from jax.experimental.shard_map import shard_map
    from jax.sharding import Mesh, PartitionSpec

    from concourse import mybir as _mybir
    from concourse.bass2jax import (_bass_exec_p, install_neuronx_cc_hook,
                                    partition_id_tensor)

    nc = get_program()
    install_neuronx_cc_hook()
    partition_name = (nc.partition_id_tensor.name
                      if nc.partition_id_tensor else None)
    in_names, out_names, out_avals, zero_outs = [], [], [], []
    for alloc in nc.m.functions[0].allocations:
        if not isinstance(alloc, _mybir.MemoryLocationSet):
            continue
        name = alloc.memorylocations[0].name
        if alloc.kind == "ExternalInput":
            if name != partition_name:
                in_names.append(name)
        elif alloc.kind == "ExternalOutput":
            shape = tuple(alloc.tensor_shape)
            dtype = _mybir.dt.np(alloc.dtype)
            out_names.append(name)
            out_avals.append(jax.core.ShapedArray(shape, dtype))
            zero_outs.append(np.zeros(shape, dtype))
    n_params = len(in_names)
    all_in_names = list(in_names) + list(out_names)
    if partition_name is not None:
        all_in_names.append(partition_name)

    def _body(*args):
        operands = list(args)
        if partition_name is not None:
            operands.append(partition_id_tensor())
        outs = _bass_exec_p.bind(
            *operands,
            out_avals=tuple(out_avals),
            in_names=tuple(all_in_names),
            out_names=tuple(out_names),
            lowering_input_output_aliases=(),
            sim_require_finite=True,
            sim_require_nnan=True,
            nc=nc,
        )
        return tuple(outs)

    devices = jax.devices()[:NCORES]
    mesh = Mesh(np.asarray(devices), ("core",))
    n_outs = len(out_names)
    sharded = jax.jit(
        shard_map(_body, mesh=mesh,
                  in_specs=(PartitionSpec("core"),) * (n_params + n_outs),
                  out_specs=(PartitionSpec("core"),) * n_outs,
                  check_rep=False),
        keep_unused=True,
    )
    _EXEC = (sharded, in_names, out_names, out_avals, zero_outs, mesh)
    return _EXEC


def run_sharded(in_maps, reps=1):
    """Execute on 8 cores; returns (per-core results, [wall_ns per rep])."""
    import time as _time

    import jax

    sharded, in_names, out_names, out_avals, zero_outs, mesh = _make_exec()
    from jax.sharding import NamedSharding, PartitionSpec
    shard = NamedSharding(mesh, PartitionSpec("core"))
    concat_in = [
        jax.device_put(
            np.concatenate([np.asarray(in_maps[c][n])
                            for c in range(NCORES)], axis=0), shard)
        for n in in_names
    ]
    concat_zeros = [
        jax.device_put(
            np.zeros((NCORES * z.shape[0], *z.shape[1:]), z.dtype), shard)
        for z in zero_outs
    ]
    args = concat_in + concat_zeros
    jax.block_until_ready(args)
    out_arrs = None
    times = []
    for _ in range(max(1, reps)):
        t0 = _time.perf_counter()
        out_arrs = sharded(*args)
        jax.block_until_ready(out_arrs)
        times.append((_time.perf_counter() - t0) * 1e9)
    results = [
        {
            name: np.asarray(out_arrs[i]).reshape(
                NCORES, *out_avals[i].shape)[c]
            for i, name in enumerate(out_names)
        }
        for c in range(NCORES)
    ]
    return results, times


def kernel(query_fea, a_nei, vec_nei, fea_emb, nei_mask,
           W1, b1, W2, b2, W3, b3, trace=False, reps=1):
    global LAST_RESULT
    args = [np.asarray(x) for x in (query_fea, a_nei, vec_nei, fea_emb,
                                    nei_mask, W1, b1, W2, b2, W3, b3)]
    in_maps = [_prep_core(c, *args) for c in range(NCORES)]
    results, times = run_sharded(in_maps, reps=reps)
    LAST_RESULT = {"times_ns": times}
    ys = [results[c]["y"].reshape(P_PER) for c in range(NCORES)]
    return np.concatenate(ys).reshape(B, A, 1).astype(np.float32)
